# revision 22
# baseline (speedup 1.0000x reference)
"""Trainium2 Bass kernel for CRFSegmentationModel (conv backbone + CRF Viterbi).

Sharding: batch 16 -> 8 cores x 2 samples each (pure data parallelism).

Per-core pipeline (v2):
  conv1(3x3 SAME 3->256)+relu and conv2(1x1 256->21) as PE matmuls (bias via
  DVE add from PSUM); emissions stored to HBM in [t, n] layout.

  Viterbi parallelized over K=64 chunks x 2 samples = 128 chains (one SBUF
  partition each). Chunks warm up WU=16 steps from a magnitude-matched flat
  init (pass-1 gain probes); at the reference's fp32 magnitude this
  reproduces its argmax decisions (incl. rounding-collapsed ties).

  Pass 2 records per-step backpointers via a 16-slot tmp ring; each 8-step
  batch is split: slots 0-3 -> DVE fused DKEY (key=(tmp-M)*2^38-j, max-reduce
  gives -argmax with first-index tie-break); slots 4-7 -> Pool/GpSimd
  (d=(tmp-M)-j*2^-38 via two subs, max-reduce; values scaled 2^-38).

  Chunk-boundary tags come from fwd score (chain c's final state) + bwd
  score (next chain's 16-step backward warmup, run on Pool during pass 1):
  argmax of fwd+bwd at the boundary is the Viterbi path tag there (max
  marginal; exact when the optimum is unique). The per-chunk backtrack then
  walks hist with a known one-hot tag: 3 small DVE ops per step.
"""
import numpy as np

import concourse.bacc as bacc
import concourse.mybir as mybir
from concourse.bass_types import AP
from concourse.tile import TileContext
from concourse import bass_utils

F32 = mybir.dt.float32
AT = mybir.AluOpType
AX = mybir.AxisListType

B, C_IN, H, W_IMG = 16, 3, 128, 128
HID, C = 256, 21
L = H * W_IMG
NCORES = 8
BL = B // NCORES

K = 64            # chunks per sample
S = L // K        # 256
WU = 16           # pass-2 warmup
BWU = 16          # backward (boundary) warmup
P1W = 12          # pass-1 warmup
P1G = 24          # pass-1 gain span
CH = BL * K       # 128 chains
F = C * C
BIG = float(2.0 ** 38)
INV38 = float(2.0 ** -38)
EMPAD = WU - 1                    # rows for t<0
EMLEN = EMPAD + L + 8             # + tail pad rows
ULEN = WU + S                     # em steps per chain
RS = 16                           # tmp ring slots (2 batches in flight)
LAG = 4                           # batches between Pool subs and DVE reduce
NDB = 7                           # dbuf regions (WAR decoupling)
DSL = 0                           # all ring slots via Pool subtract route

USE_POOL = False

_CACHE = {}
LAST_EXEC_NS = None


def _register_dve_ops():
    """Runtime-register the fused DKEY DVE op (idempotent, self-contained)."""
    import concourse.dve_ops as D
    from concourse.dve_spec import (Spec, Src0, Src1, C0, C1, Idx, SubIdx,
                                    lower, _has_src1)
    from concourse.dve_uop import DveOpSpec
    from concourse.dve_table_gen import dve_ver_for
    if "ANT_DKEY" in D._SUB_OPCODE_FOR_NAME:
        return {o.name: o for o in D.OPS}

    from concourse.dve_spec import Zero, eq, select

    def dkey_ref(in0, in1, c0, c1, c2):
        jj = np.arange(in0.shape[2], dtype=np.float32)[None, None, :]
        return ((in0.astype(np.float32) - in1.astype(np.float32))
                * np.float32(c1) - jj).astype(np.float32)

    def bsel_ref(in0, in1, c0, c1, c2):
        b = np.where(in1 == np.float32(c0), in0.astype(np.float32),
                     np.float32(0.0)).astype(np.float32)
        return b, b.reshape(b.shape[0], -1).sum(axis=-1, keepdims=True)

    jterm = Idx - SubIdx * C0
    specs = [
        ("ANT_DKEY", Spec(body=(Src0 - Src1) * C1 - jterm, reference=dkey_ref),
         True),
        ("ANT_BSEL", Spec(body=select(eq(Src1, C0), Src0, Zero),
                          accum=__import__("operator").add, reference=bsel_ref),
         False),
    ]
    ver = dve_ver_for("TRN2")
    for name, spec, subdim in specs:
        opcode = max(D._SUB_OPCODE_FOR_NAME.values()) + 1
        D._SUB_OPCODE_FOR_NAME[name] = opcode
        compiled = DveOpSpec(name=name, opcode=opcode, uops=lower(spec, ver=ver),
                             rd1_en=_has_src1(spec))
        op = D.DveOp(name, spec, subdim=subdim, uops_sha={ver: compiled.sha(ver)})
        D._COMPILE_CACHE[(name, ver)] = compiled
        D.OPS.append(op)
        D.CUSTOM_DVE_SPECS[name] = spec
    assert max(D._SUB_OPCODE_FOR_NAME.values()) < 0x20
    return {o.name: o for o in D.OPS}


def _r3(ap, inner=C):
    return ap.rearrange("p (a b) -> p a b", b=inner)


def _build():
    if "nc" in _CACHE:
        return _CACHE["nc"]
    ops = _register_dve_ops()
    DKEY = ops["ANT_DKEY"]
    BSEL = ops["ANT_BSEL"]
    nc = bacc.Bacc("TRN2", target_bir_lowering=False, debug=False, num_devices=1)
    gp_eng = "gpsimd" if USE_POOL else "vector"

    x_d = nc.dram_tensor("x", (BL, C_IN, H, W_IMG), F32, kind="ExternalInput").ap()
    w1_d = nc.dram_tensor("w1i", (27, HID), F32, kind="ExternalInput").ap()
    b1_d = nc.dram_tensor("b1", (128, 2), F32, kind="ExternalInput").ap()
    w2_d = nc.dram_tensor("w2e", (128, 2 * C), F32, kind="ExternalInput").ap()
    b2r4_d = nc.dram_tensor("b2rep4", (128, 4 * C), F32, kind="ExternalInput").ap()
    startrep_d = nc.dram_tensor("start_rep", (BL, C), F32, kind="ExternalInput").ap()
    endrep_d = nc.dram_tensor("end_rep", (BL, C), F32, kind="ExternalInput").ap()
    transrep_d = nc.dram_tensor("transrep", (CH, F), F32, kind="ExternalInput").ap()
    transflat_d = nc.dram_tensor("transflat", (CH, F), F32, kind="ExternalInput").ap()
    i38rep_d = nc.dram_tensor("i38rep", (CH, F), F32, kind="ExternalInput").ap()
    negi21_d = nc.dram_tensor("negi21ch", (CH, C), F32, kind="ExternalInput").ap()
    negi38_d = nc.dram_tensor("negi38ch", (CH, C), F32, kind="ExternalInput").ap()
    zeros_d = nc.dram_tensor("zrow", (1, (S + 1) * C), F32, kind="ExternalInput").ap()

    tags_d = nc.dram_tensor("tags", (BL, L), F32, kind="ExternalOutput").ap()

    em_d = nc.dram_tensor("em_hbm", (BL, EMLEN, C), F32, kind="Internal").ap()
    bounce_d = nc.dram_tensor("bounce", (1, 8), F32, kind="Internal").ap()
    gsum_d = nc.dram_tensor("gsum", (2, CH), F32, kind="Internal").ap()

    em_flat = [em_d[b].rearrange("t n -> (t n)") for b in range(BL)]

    with TileContext(nc) as tc:
        # outer pool: tiles alive across conv + viterbi
        with tc.tile_pool(name="empool", bufs=1) as ep:
            emt = ep.tile([CH, ULEN * C], F32)
            transrep = ep.tile([CH, F], F32)
            transflat = ep.tile([CH, F], F32)
            i38rep = ep.tile([CH, F], F32)
            negi21 = ep.tile([CH, C], F32)
            negi38 = ep.tile([CH, C], F32)
            startrep = ep.tile([BL, C], F32)
            endrep = ep.tile([BL, C], F32)
            b2r4sb = ep.tile([128, 4 * C], F32)

            nc.sync.dma_start(transrep[:], transrep_d[:])
            nc.sync.dma_start(transflat[:], transflat_d[:])
            nc.sync.dma_start(i38rep[:], i38rep_d[:])
            nc.sync.dma_start(negi21[:], negi21_d[:])
            nc.sync.dma_start(negi38[:], negi38_d[:])
            nc.sync.dma_start(startrep[:], startrep_d[:])
            nc.sync.dma_start(endrep[:], endrep_d[:])
            nc.sync.dma_start(b2r4sb[:], b2r4_d[:])

            # ====================== conv backbone ======================
            with tc.tile_pool(name="convpool", bufs=1) as cp, \
                 tc.tile_pool(name="convwork", bufs=4) as cw, \
                 tc.tile_pool(name="convpsum", bufs=4, space="PSUM") as cpp:
                xpad = cp.tile([C_IN, 130 * 130], F32)
                im2col = cp.tile([27, L], F32)
                w1sb = cp.tile([27, HID], F32)
                b1sb = cp.tile([128, 2], F32)
                w2sb = cp.tile([128, 2 * C], F32)
                zrow = cp.tile([1, (S + 1) * C], F32)

                nc.sync.dma_start(w1sb[:], w1_d[:])
                nc.sync.dma_start(b1sb[:], b1_d[:])
                nc.sync.dma_start(w2sb[:], w2_d[:])
                nc.sync.dma_start(zrow[:], zeros_d[:])

                # zero only the SAME-padding border (interior overwritten per b)
                xp3 = xpad[:].rearrange("p (y xx) -> p y xx", xx=130)
                nc.vector.memset(xp3[:, 0:1, :], 0.0)
                nc.vector.memset(xp3[:, 129:130, :], 0.0)
                nc.vector.memset(xp3[:, 1:129, 0:1], 0.0)
                nc.vector.memset(xp3[:, 1:129, 129:130], 0.0)

                for b in range(BL):
                    nc.sync.dma_start(xp3[:, 1:129, 1:129], x_d[b])
                    for dy in range(3):
                        for dx in range(3):
                            r0 = (dy * 3 + dx) * 3
                            nc.sync.dma_start(
                                im2col[r0:r0 + 3, :].rearrange(
                                    "p (y xx) -> p y xx", xx=128),
                                xp3[:, dy:dy + 128, dx:dx + 128])
                    # front/tail zero pads of em
                    nc.sync.dma_start(
                        AP(tensor=em_flat[b].tensor, offset=b * EMLEN * C,
                           ap=[[0, 1], [1, EMPAD * C]]),
                        zrow[:, 0:EMPAD * C])
                    nc.sync.dma_start(
                        AP(tensor=em_flat[b].tensor,
                           offset=b * EMLEN * C + (EMPAD + L) * C,
                           ap=[[0, 1], [1, (EMLEN - EMPAD - L) * C]]),
                        zrow[:, 0:(EMLEN - EMPAD - L) * C])

                    for tt in range(0, L, 512):
                        hid0 = cw.tile([128, 512], F32, tag="hid0")
                        hid1 = cw.tile([128, 512], F32, tag="hid1")
                        for hti, hid in ((0, hid0), (1, hid1)):
                            ps = cpp.tile([128, 512], F32, tag="psc1")
                            nc.tensor.matmul(
                                ps[:], w1sb[:, hti * 128:(hti + 1) * 128],
                                im2col[:, tt:tt + 512], start=True, stop=True)
                            nc.scalar.activation(
                                hid[:], ps[:], mybir.ActivationFunctionType.Relu,
                                bias=b1sb[:, hti:hti + 1], scale=1.0)
                        ps2 = cpp.tile([128, 4 * C], F32, tag="psc2")
                        for blk in range(4):
                            t0 = blk * 128
                            o = ps2[:, blk * C:(blk + 1) * C]
                            nc.tensor.matmul(o, hid0[:, t0:t0 + 128], w2sb[:, 0:C],
                                             start=True, stop=False)
                            nc.tensor.matmul(o, hid1[:, t0:t0 + 128], w2sb[:, C:2 * C],
                                             start=False, stop=True)
                        emst = cw.tile([128, 4 * C], F32, tag="emst")
                        nc.vector.tensor_tensor(out=emst[:], in0=ps2[:],
                                                in1=b2r4sb[:], op=AT.add)
                        nc.sync.dma_start(
                            em_d[b, EMPAD + tt:EMPAD + tt + 512]
                            .rearrange("(blk p) n -> p blk n", p=128),
                            emst[:].rearrange("p (blk n) -> p blk n", n=C))

                    # per-sample chunk rows: chain (b,c) covers em rows
                    # [cS, cS+ULEN) of em_d[b] (t = cS + u - EMPAD)
                    nc.sync.dma_start(
                        emt[b * K:(b + 1) * K, :],
                        AP(tensor=em_flat[b].tensor, offset=b * EMLEN * C,
                           ap=[[S * C, K], [1, ULEN * C]]))

            # ====================== viterbi ======================
            with tc.tile_pool(name="vit", bufs=1) as vp:
                score = vp.tile([CH, C], F32)
                m_work = vp.tile([CH, C], F32)
                tmp_ring = vp.tile([CH, RS * F], F32)
                m_ring = vp.tile([CH, RS * C], F32)
                dbufs = []
                for _i in range(NDB if USE_POOL else 1):
                    dbuf_i = vp.tile([CH, 8 * F], F32, tag=f"dbuf{_i}",
                                     name=f"dbuf{_i}")
                    dbufs.append(dbuf_i)
                hist = vp.tile([CH, S * C], F32)
                msum0 = vp.tile([CH, 1], F32)
                msum1 = vp.tile([CH, 1], F32)
                gp = vp.tile([1, 3 * CH], F32)
                vinit = vp.tile([CH, 1], F32)
                s0t = vp.tile([BL, C], F32)
                fs = vp.tile([BL, C], F32)
                small1 = vp.tile([CH, 1], F32)
                tagsf = vp.tile([CH, S], F32)
                # boundary decode
                ub = vp.tile([CH, C], F32)
                vb = vp.tile([CH, C], F32)
                tmpb = vp.tile([CH, F], F32)
                vsh = vp.tile([CH, C], F32)
                tot = vp.tile([CH, C], F32)
                keyb = vp.tile([CH, C], F32)
                oh = vp.tile([CH, C], F32)
                selb = vp.tile([CH, C], F32)

                gpe = getattr(nc, gp_eng)

                # score0 = em[t=0] + start
                nc.sync.dma_start(s0t[:], em_d[:, EMPAD, :])
                nc.vector.tensor_tensor(out=s0t[:], in0=s0t[:], in1=startrep[:],
                                        op=AT.add)

                def emsl(u):
                    return emt[:, u * C:(u + 1) * C]

                def step(u_em, m_dst, tmp_cur):
                    nc.vector.tensor_tensor(
                        out=_r3(tmp_cur),
                        in0=score[:].unsqueeze(1).broadcast_to((CH, C, C)),
                        in1=_r3(transrep[:]), op=AT.add)
                    nc.vector.tensor_reduce(out=m_dst, in_=_r3(tmp_cur),
                                            axis=AX.X, op=AT.max)
                    nc.vector.tensor_tensor(out=score[:], in0=m_dst,
                                            in1=emsl(u_em), op=AT.add)

                # ---------- backward boundary warmup ----------
                # v_t[p] = max_n(trans[p,n] + u_{t+1}[n]); u_t = em_t + v_t
                # ends with vb = v at t = cS (chain's first position).
                nc.vector.tensor_copy(ub[:], emsl(WU + BWU - 1))
                for kk in range(BWU - 1, -1, -1):
                    nc.vector.tensor_tensor(
                        out=_r3(tmpb[:]),
                        in0=ub[:].unsqueeze(1).broadcast_to((CH, C, C)),
                        in1=_r3(transflat[:]), op=AT.add)
                    nc.vector.tensor_reduce(out=vb[:], in_=_r3(tmpb[:]),
                                            axis=AX.X, op=AT.max)
                    if kk > 0:
                        nc.vector.tensor_tensor(out=ub[:], in0=vb[:],
                                                in1=emsl(WU + kk - 1), op=AT.add)

                # ---------- pass 1: gain probe ----------
                nc.vector.memset(score[:], 0.0)
                for s in range(P1W + P1G):
                    step(WU - P1W + s, m_work[:], tmp_ring[:, 0:F])
                    if s == P1W - 1:
                        nc.vector.tensor_reduce(out=msum0[:], in_=score[:],
                                                axis=AX.X, op=AT.add)
                nc.vector.tensor_reduce(out=msum1[:], in_=score[:], axis=AX.X,
                                        op=AT.add)

                nc.sync.dma_start(AP(tensor=gsum_d.tensor, offset=0,
                                     ap=[[1, CH], [1, 1]]), msum0[:])
                nc.sync.dma_start(AP(tensor=gsum_d.tensor, offset=CH,
                                     ap=[[1, CH], [1, 1]]), msum1[:])
                nc.sync.dma_start(gp[0:1, 0:2 * CH],
                                  gsum_d[:].rearrange("a b -> (a b)").unsqueeze(0))
                # g*S per chain -> gp[0, 2CH:3CH]
                nc.vector.tensor_tensor(out=gp[0:1, 2 * CH:3 * CH],
                                        in0=gp[0:1, CH:2 * CH],
                                        in1=gp[0:1, 0:CH], op=AT.subtract)
                nc.vector.tensor_scalar(out=gp[0:1, 2 * CH:3 * CH],
                                        in0=gp[0:1, 2 * CH:3 * CH],
                                        scalar1=float(S) / (C * P1G), scalar2=None,
                                        op0=AT.mult)
                # exclusive prefix into gp[0, 0:CH] (ping-pong)
                pfa = vp.tile([1, CH], F32)
                pfb = vp.tile([1, CH], F32)
                nc.vector.memset(pfa[:], 0.0)
                nc.vector.tensor_copy(pfa[0:1, 1:K], gp[0:1, 2 * CH:2 * CH + K - 1])
                nc.vector.tensor_copy(pfa[0:1, K + 1:2 * K],
                                      gp[0:1, 2 * CH + K:2 * CH + 2 * K - 1])
                cur, nxt = pfa, pfb
                for sh in (1, 2, 4, 8, 16, 32):
                    for h0 in (0, K):
                        nc.vector.tensor_copy(nxt[0:1, h0:h0 + sh],
                                              cur[0:1, h0:h0 + sh])
                        nc.vector.tensor_tensor(
                            out=nxt[0:1, h0 + sh:h0 + K],
                            in0=cur[0:1, h0 + sh:h0 + K],
                            in1=cur[0:1, h0:h0 + K - sh], op=AT.add)
                    cur, nxt = nxt, cur
                nc.vector.tensor_copy(gp[0:1, 0:CH], cur[0:1, 0:CH])
                # per-sample base mean(score0)/C
                nc.vector.tensor_reduce(out=small1[0:BL, :], in_=s0t[:],
                                        axis=AX.X, op=AT.add)
                nc.vector.tensor_scalar(out=small1[0:BL, :], in0=small1[0:BL, :],
                                        scalar1=1.0 / C, scalar2=None, op0=AT.mult)
                nc.sync.dma_start(bounce_d[0:1, 0:1], small1[0:1, :])
                nc.sync.dma_start(bounce_d[0:1, 1:2], small1[1:2, :])
                base2 = vp.tile([1, 2], F32)
                nc.sync.dma_start(base2[:], bounce_d[0:1, 0:2])
                nc.vector.tensor_scalar(out=gp[0:1, 0:K], in0=gp[0:1, 0:K],
                                        scalar1=base2[0:1, 0:1], scalar2=None,
                                        op0=AT.add)
                nc.vector.tensor_scalar(out=gp[0:1, K:2 * K], in0=gp[0:1, K:2 * K],
                                        scalar1=base2[0:1, 1:2], scalar2=None,
                                        op0=AT.add)
                nc.sync.dma_start(gsum_d[0:1, :], gp[0:1, 0:CH])
                nc.sync.dma_start(vinit[:], AP(tensor=gsum_d.tensor, offset=0,
                                               ap=[[1, CH], [1, 1]]))

                # ---------- pass 2 ----------
                nc.vector.memset(score[:], 0.0)
                nc.vector.tensor_scalar(out=score[:], in0=score[:],
                                        scalar1=vinit[:, :], scalar2=None,
                                        op0=AT.add)
                pending = []

                def drain_piece(n=1):
                    for _ in range(n):
                        if pending:
                            pending.pop(0)()

                for s in range(WU + S):
                    if s == WU:
                        # chunk 0 records from the exact t=0 state
                        nc.sync.dma_start(score[0:1, :], s0t[0:1, :])
                        nc.sync.dma_start(score[K:K + 1, :], s0t[1:2, :])
                    rec = s >= WU
                    r = s - WU
                    slot = (r % RS) if rec else (RS - 1)
                    step(s, m_ring[:, slot * C:(slot + 1) * C] if rec else m_work[:],
                         tmp_ring[:, slot * F:(slot + 1) * F])
                    drain_piece(3 if r % 8 == 7 else 2)
                    if s == WU + S - 2:
                        nc.sync.dma_start(fs[0:1, :], score[K - 1:K, :])
                        nc.sync.dma_start(fs[1:2, :], score[CH - 1:CH, :])
                    if USE_POOL and rec and (r % 8 == 0) and r >= 8 * LAG:
                        # lagged max-reduce of batch (r//8 - LAG), emitted at
                        # the first step after a boundary: the scheduler's
                        # (optimistic-Pool) sim then latches its Pool-sem wait
                        # at the prior batch's d2, which real timing meets.
                        bl2 = r // 8 - LAG
                        rp = bl2 * 8
                        nc.vector.tensor_reduce(
                            out=hist[:, rp * C:(rp + 8) * C],
                            in_=_r3(dbufs[bl2 % NDB][:]), axis=AX.X,
                            op=AT.max)
                    if rec and (r % 8 == 7):
                        r0 = r - 7
                        bi = r0 // 8           # batch index
                        sl0 = r0 % RS          # ring region start (0 or 8)
                        if USE_POOL:
                            db = dbufs[bi % NDB][:]
                            # Pool: d = (tmp - m) - j*2^-38 (two subs); the
                            # max-reduce runs on DVE two batches later.
                            gpe.tensor_tensor(
                                out=_r3(db),
                                in0=_r3(tmp_ring[:, sl0 * F:(sl0 + 8) * F]),
                                in1=m_ring[:, sl0 * C:(sl0 + 8) * C]
                                    .unsqueeze(2).broadcast_to((CH, 8 * C, C)),
                                op=AT.subtract)
                            gpe.tensor_tensor(
                                out=db.rearrange("p (s f) -> p s f", f=F),
                                in0=db.rearrange("p (s f) -> p s f", f=F),
                                in1=i38rep[:].unsqueeze(1)
                                    .broadcast_to((CH, 8, F)),
                                op=AT.subtract)
                        else:
                            # fused key=(tmp-m)*2^38-j then max-reduce, as
                            # per-slot pieces drained into the next batch's
                            # dependency bubbles; hist post-scaled to 2^-38.
                            def mk_dkey(pz, sl0=sl0):
                                def f():
                                    nc.vector._custom_dve(
                                        DKEY,
                                        out=_r3(dbufs[0][:, pz * F:(pz + 1) * F]),
                                        in0=_r3(tmp_ring[:, (sl0 + pz) * F:
                                                         (sl0 + pz + 1) * F]),
                                        in1=m_ring[:, (sl0 + pz) * C:
                                                   (sl0 + pz + 1) * C]
                                            .unsqueeze(2)
                                            .broadcast_to((CH, C, C)),
                                        s0=float(C), s1=BIG)
                                return f

                            def mk_tr(pz, r0=r0):
                                def f():
                                    nc.vector.tensor_reduce(
                                        out=hist[:, (r0 + pz) * C:
                                                 (r0 + pz + 1) * C],
                                        in_=_r3(dbufs[0][:, pz * F:(pz + 1) * F]),
                                        axis=AX.X, op=AT.max)
                                return f

                            def mk_ts(r0=r0):
                                def f():
                                    nc.vector.tensor_scalar(
                                        out=hist[:, r0 * C:(r0 + 8) * C],
                                        in0=hist[:, r0 * C:(r0 + 8) * C],
                                        scalar1=float(INV38), scalar2=None,
                                        op0=AT.mult)
                                return f
                            for pz in range(8):
                                pending.append(mk_dkey(pz))
                                pending.append(mk_tr(pz))
                            pending.append(mk_ts())
                drain_piece(32)
                if USE_POOL:
                    # trailing reduces (last LAG batches)
                    for bi in range(S // 8 - LAG, S // 8):
                        rp = bi * 8
                        nc.vector.tensor_reduce(
                            out=hist[:, rp * C:(rp + 8) * C],
                            in_=_r3(dbufs[bi % NDB][:]), axis=AX.X,
                            op=AT.max)

                # ---------- boundary tags ----------
                # identity-fix hist row S-1 of last chain of each sample
                # (r = S-1 is a Pool slot -> 2^-38 units)
                nc.sync.dma_start(hist[K - 1:K, (S - 1) * C:], negi38_d[0:1, :])
                nc.sync.dma_start(hist[CH - 1:CH, (S - 1) * C:], negi38_d[0:1, :])
                # vsh[c] = vb[c+1]; last chain of each sample uses end_trans.
                # score rows for last chains <- fs (score at position L-1).
                nc.sync.dma_start(vsh[0:K - 1, :], vb[1:K, :])
                nc.sync.dma_start(vsh[K:CH - 1, :], vb[K + 1:CH, :])
                nc.sync.dma_start(vsh[K - 1:K, :], endrep_d[0:1, :])
                nc.sync.dma_start(vsh[CH - 1:CH, :], endrep_d[1:2, :])
                nc.sync.dma_start(score[K - 1:K, :], fs[0:1, :])
                nc.sync.dma_start(score[CH - 1:CH, :], fs[1:2, :])
                nc.vector.tensor_tensor(out=tot[:], in0=score[:], in1=vsh[:],
                                        op=AT.add)
                # argmax -> one-hot (first-index tie-break via -iota packing)
                nc.vector.tensor_reduce(out=small1[:], in_=tot[:], axis=AX.X,
                                        op=AT.max)
                nc.vector.tensor_scalar(out=keyb[:], in0=tot[:],
                                        scalar1=small1[:, :], scalar2=BIG,
                                        op0=AT.subtract, op1=AT.mult)
                nc.vector.tensor_tensor(out=keyb[:], in0=keyb[:], in1=negi21[:],
                                        op=AT.add)
                nc.vector.tensor_reduce(out=small1[:], in_=keyb[:], axis=AX.X,
                                        op=AT.max)
                t0v = vp.tile([CH, 1], F32)
                nc.vector.tensor_scalar(out=t0v[:], in0=small1[:],
                                        scalar1=float(INV38), scalar2=None,
                                        op0=AT.mult)

                # ---------- backtrack with known boundary tag ----------
                for r in range(S - 1, -1, -1):
                    prev = t0v[:, 0:1] if r == S - 1 else tagsf[:, r + 1:r + 2]
                    nc.vector._custom_dve(
                        BSEL, out=selb[:], in0=hist[:, r * C:(r + 1) * C],
                        in1=negi38[:], s0=prev,
                        accum_out=tagsf[:, r:r + 1])

                # ---------- output ----------
                nc.vector.tensor_scalar(out=tagsf[:], in0=tagsf[:],
                                        scalar1=-BIG, scalar2=None, op0=AT.mult)
                for b in range(BL):
                    nc.sync.dma_start(
                        tags_d[b].rearrange("(c r) -> c r", r=S),
                        tagsf[b * K:(b + 1) * K, :])

    nc.compile()
    _CACHE["nc"] = nc
    return nc


def _consts():
    if "consts" not in _CACHE:
        iota = np.arange(C, dtype=np.float32)
        i38rep = np.tile(np.tile(iota * np.float32(INV38), C)[None, :], (CH, 1))
        negi21 = np.tile(-iota[None, :], (CH, 1))
        negi38 = np.tile((-iota * np.float32(INV38))[None, :], (CH, 1))
        zrow = np.zeros((1, (S + 1) * C), np.float32)
        _CACHE["consts"] = (i38rep.astype(np.float32), negi21.astype(np.float32),
                            negi38.astype(np.float32), zrow)
    return _CACHE["consts"]


def kernel(x, conv1_w, conv1_b, conv2_w, conv2_b, start_trans, end_trans, trans):
    x = np.ascontiguousarray(np.asarray(x, np.float32))
    nc = _build()
    i38rep, negi21, negi38, zrow = _consts()

    trans = np.asarray(trans, np.float32)
    transrep = np.tile(np.ascontiguousarray(trans.T).reshape(1, F),
                       (CH, 1)).astype(np.float32)
    transflat = np.tile(trans.reshape(1, F), (CH, 1)).astype(np.float32)
    w1i = np.ascontiguousarray(
        np.asarray(conv1_w, np.float32).transpose(2, 3, 1, 0).reshape(27, HID))
    b1 = np.ascontiguousarray(np.asarray(conv1_b, np.float32).reshape(2, 128).T)
    w2e = np.ascontiguousarray(
        np.asarray(conv2_w, np.float32).reshape(C, HID).T.reshape(2, 128, C)
        .transpose(1, 0, 2).reshape(128, 2 * C))
    b2rep4 = np.tile(np.asarray(conv2_b, np.float32).reshape(1, C),
                     (128, 4)).astype(np.float32)
    startrep = np.tile(np.asarray(start_trans, np.float32).reshape(1, C), (BL, 1))
    endrep = np.tile(np.asarray(end_trans, np.float32).reshape(1, C), (BL, 1))

    in_maps = []
    for core in range(NCORES):
        in_maps.append({
            "x": np.ascontiguousarray(x[core * BL:(core + 1) * BL]),
            "w1i": w1i, "b1": b1, "w2e": w2e, "b2rep4": b2rep4,
            "start_rep": startrep, "end_rep": endrep,
            "transrep": transrep, "transflat": transflat, "i38rep": i38rep,
            "negi21ch": negi21, "negi38ch": negi38, "zrow": zrow,
        })
    import os
    trace = bool(os.environ.get("BASS_TRACE_RUN"))
    res = bass_utils.run_bass_kernel_spmd(nc, in_maps, core_ids=list(range(NCORES)),
                                          trace=trace)
    global LAST_EXEC_NS
    LAST_EXEC_NS = res.exec_time_ns
    out = np.concatenate([r["tags"] for r in res.results], axis=0)
    return np.rint(out).astype(np.int32).reshape(B, H, W_IMG)
